# revision 2
# baseline (speedup 1.0000x reference)
"""FCPlanenet Trainium2 kernel v3 (8-core data-parallel over batch).

All matmuls bf16 (weights + activations; fp32 PSUM accumulate).
Per batch (T=8192 points, feature-major [feat, T] on chip):
  net0 = p @ Wpos + bpos            [256, T]   (K=3, quad-packed)
  net1 = relu(net0) @ W0 + b0       [128, T]   (K=256)  "L0"
  net2/net3 via pooled layers L1/L2; D = W3a matmul, max only.

Engine split per batch (every phase keeps ACT and DVE both fed):
  - all layers: true-relu drains (bias = vec_k) split ACT/DVE per
    supertile (DVE via tensor_scalar add+max), bf16 out
  - pooled maxes: DVE bf16 tensor_tensor max tree per layer -> m_k
    (2x DVE perf mode on bf16), vec_{k+1} = Wb^T m_k + b_{k+1} (PE matvec)
  - D: no drain; DVE reduce_max straight from PSUM; s = max + vec3
  - pos(b+1) supertiles are the filler work during L1/L2/D of batch b
  - head: 5 tiny bf16 matvecs (PE) + DVE relu steps

tensor_tensor_reduce is avoided: it hard-crashes the device
(NRT_EXEC_UNIT_UNRECOVERABLE) despite passing CoreSim.
"""

import os
import sys

import numpy as np

for _p in ("/opt/trn_rl_repo", "/root/.axon_site/_ro/trn_rl_repo"):
    if os.path.isdir(_p) and _p not in sys.path:
        sys.path.insert(0, _p)

import concourse.bass as bass  # noqa: E402
import concourse.tile as tile  # noqa: E402
from concourse import bacc, mybir  # noqa: E402
from concourse.bass_utils import run_bass_kernel_spmd  # noqa: E402

F32 = mybir.dt.float32
BF16 = mybir.dt.bfloat16
AX = mybir.AxisListType.X
MAX = mybir.AluOpType.max
ADD = mybir.AluOpType.add
MULT = mybir.AluOpType.mult
RELU = mybir.ActivationFunctionType.Relu
IDENT = mybir.ActivationFunctionType.Identity

NCORES = 8
B = 32
T = 8192
BPC = B // NCORES  # batches per core
NQ = 4             # point quads (K=3 matmul row-packing)
QT = T // NQ       # 2048 points per quad
NCH = 512          # matmul free-dim chunk (one PSUM bank)
NSUP = 1024        # drain supertile (2 banks)
NST = T // NSUP    # 8 supertiles per layer

# bias tile columns
BC_BPOS_A, BC_BPOS_B = 0, 1
BC_B0, BC_B1, BC_B2, BC_B3 = 2, 3, 4, 5
BC_BC, BC_BM0, BC_BM1, BC_BM2, BC_BP = 6, 7, 8, 9, 10

# wm tile blocks of 128 cols: w0a w0b w1a w1b w2a w2b w3a w3b wc wm0 wm1 wm2 wp
WM_COLS = 13 * 128 + 16

# pos supertiles drained on DVE instead of ACT (engine balance): (qp, s, h)
POS_DVE = ((0, 3, 0), (1, 3, 0))
# L-layer supertiles drained on DVE (engine balance): st indices
L_DVE = {0: (2, 6), 1: (2, 6), 2: (2, 6)}


def _emit_core_program(tc, nc, pt_d, wpos_d, wm_d, bias_d, out_d, reps=1):
    ctx_pools = []

    def pool(name, bufs, space="SBUF"):
        p = tc.alloc_tile_pool(name=name, bufs=bufs, space=space)
        ctx_pools.append(p)
        return p

    const = pool("const", 1)
    ptp = pool("ptp", 2)
    r0p = pool("r0p", 1)
    netp = pool("netp", 2)
    treep = pool("treep", 2)
    smallp = pool("smallp", 8)
    vecp = pool("vecp", 40)
    headp = pool("headp", 2)
    psmm = pool("psmm", 3, space="PSUM")
    psvp = pool("psvp", 2, space="PSUM")

    # ---- constants ----
    wpos_sb = const.tile([99, 256], BF16, name="wpos_sb")
    for q in range(NQ):
        nc.sync.dma_start(wpos_sb[32 * q:32 * q + 3, :], wpos_d[q])
    bias_sb = const.tile([128, 16], F32, name="bias_sb")
    nc.sync.dma_start(bias_sb[:], bias_d)
    wm_sb = const.tile([128, WM_COLS], BF16, name="wm_sb")
    _wm_loaded = [False]

    def load_wm():
        if not _wm_loaded[0]:
            nc.sync.dma_start(wm_sb[:, 0:256], wm_d[:, 0:256])
            nc.sync.dma_start(wm_sb[:, 256:], wm_d[:, 256:])
            _wm_loaded[0] = True

    def W(i):
        return wm_sb[:, 128 * i:128 * (i + 1)]

    def bcol(i):
        return bias_sb[:, i:i + 1]

    def pos_tasks(b, pt_sb, r0):
        """16 supertile tasks for the pos layer of batch b."""
        tasks = []
        for qp in range(2):
            for s in range(4):
                for h in range(2):
                    def t(qp=qp, s=s, h=h):
                        ps = psmm.tile([128, NSUP], F32, tag="mm", name="ps_pos")
                        for jq in range(2):
                            q = 2 * qp + jq
                            nc.tensor.matmul(
                                ps[:, NCH * jq:NCH * (jq + 1)],
                                wpos_sb[32 * q:32 * q + 3, 128 * h:128 * (h + 1)],
                                pt_sb[32 * q:32 * q + 3, NCH * s:NCH * (s + 1)],
                                start=True, stop=True,
                                tile_position=(32 * q, 0),
                            )
                        g0 = 8 * qp + s
                        dst = (r0[h].rearrange("p (g c) -> p g c", c=NCH)
                               [:, g0:g0 + 5:4, :])
                        srcv = ps.rearrange("p (g c) -> p g c", c=NCH)
                        if (qp, s, h) in POS_DVE:
                            nc.vector.tensor_scalar(dst, srcv, bcol(BC_BPOS_A + h),
                                                    0.0, op0=ADD, op1=MAX)
                        else:
                            nc.scalar.activation(dst, srcv, RELU,
                                                 bias=bcol(BC_BPOS_A + h))
                    tasks.append(t)
        return tasks

    def tree_tasks(r_out, b, li):
        """bf16 TT-max tree over r_out [128, T] -> m [128,1] bf16.
        lvl1 task i depends on r_out supertiles 2i, 2i+1 being drained."""
        u = treep.tile([128, 4 * NSUP], BF16, tag=f"u{li}", name=f"u{li}_{b}")
        res = {}

        def lvl1(i):
            def t():
                nc.vector.tensor_tensor(
                    u[:, NSUP * i:NSUP * (i + 1)],
                    r_out[:, 2 * NSUP * i:NSUP * (2 * i + 1)],
                    r_out[:, NSUP * (2 * i + 1):2 * NSUP * (i + 1)], op=MAX)
            return t

        def tail():
            # u0..u3 [128,1024] -> v1,v2 -> w -> reduce -> m [128,1] bf16
            v = treep.tile([128, 3 * NSUP], BF16, tag=f"v{li}",
                           name=f"v{li}_{b}")
            nc.vector.tensor_tensor(v[:, 0:NSUP], u[:, 0:NSUP],
                                    u[:, NSUP:2 * NSUP], op=MAX)
            nc.vector.tensor_tensor(v[:, NSUP:2 * NSUP], u[:, 2 * NSUP:3 * NSUP],
                                    u[:, 3 * NSUP:], op=MAX)
            nc.vector.tensor_tensor(v[:, 2 * NSUP:], v[:, 0:NSUP],
                                    v[:, NSUP:2 * NSUP], op=MAX)
            m = vecp.tile([128, 1], BF16, tag="vb", name=f"m{li}_{b}")
            nc.vector.reduce_max(m, v[:, 2 * NSUP:], axis=AX)
            res["m"] = m
        return [lvl1(i) for i in range(4)], tail, res

    def layer_tasks(li, b, r_in, r_out, vec_box):
        """Layer li (0..2): matmul + true-relu drain (bias=vec) -> bf16
        r_out.  Drains split ACT / DVE per L_DVE[li].  li==0 contracts
        K=256 from the pos pair r_in=(r0a, r0b)."""
        tasks = []
        for st in range(NST):
            def t(st=st):
                ps = psmm.tile([128, NSUP], F32, tag="mm", name=f"ps_l{li}")
                for j in range(2):
                    c = 2 * st + j
                    osl = ps[:, NCH * j:NCH * (j + 1)]
                    csl = slice(NCH * c, NCH * (c + 1))
                    if li == 0:
                        nc.tensor.matmul(osl, W(0), r_in[0][:, csl],
                                         start=True, stop=False)
                        nc.tensor.matmul(osl, W(1), r_in[1][:, csl],
                                         start=False, stop=True)
                    else:
                        nc.tensor.matmul(osl, W(2 * li), r_in[:, csl],
                                         start=True, stop=True)
                dsl = slice(NSUP * st, NSUP * (st + 1))
                if st in L_DVE[li]:
                    nc.vector.tensor_scalar(r_out[:, dsl], ps[:], vec_box[0],
                                            0.0, op0=ADD, op1=MAX)
                else:
                    nc.scalar.activation(r_out[:, dsl], ps[:], RELU,
                                         bias=vec_box[0])
            tasks.append(t)
        return tasks

    def d_tasks(b, r3, partD):
        """D: matmul + DVE reduce_max straight from PSUM -> partD[:, st]."""
        tasks = []
        for st in range(NST):
            def t(st=st):
                ps = psmm.tile([128, NSUP], F32, tag="mm", name="ps_d")
                for j in range(2):
                    c = 2 * st + j
                    nc.tensor.matmul(ps[:, NCH * j:NCH * (j + 1)], W(6),
                                     r3[:, NCH * c:NCH * (c + 1)],
                                     start=True, stop=True)
                nc.vector.reduce_max(partD[:, st:st + 1], ps[:], axis=AX)
            tasks.append(t)
        return tasks

    def interleave(a, bl):
        out = []
        n = max(len(a), len(bl))
        for i in range(n):
            if i < len(a):
                out.append(a[i])
            if i < len(bl):
                out.append(bl[i])
        return out

    import contextlib

    def _rep_scope():
        if reps > 1:
            return tc.For_i(0, reps, 1,
                            hint_engines=(mybir.EngineType.PE,
                                          mybir.EngineType.Activation,
                                          mybir.EngineType.DVE))
        return contextlib.nullcontext()

    with _rep_scope():

        def new_batch_state(b):
            pt_sb = ptp.tile([99, QT], BF16, tag="pt", name="pt_sb")
            for q in range(NQ):
                nc.sync.dma_start(pt_sb[32 * q:32 * q + 3, :], pt_d[b, q])
            r0a = r0p.tile([128, T], BF16, tag="r0a", name="r0a")
            r0b = r0p.tile([128, T], BF16, tag="r0b", name="r0b")
            return pt_sb, (r0a, r0b)

        def boundary(li, b, m_bf):
            """vec_{li+1} = W(2*li+3)^T m_li + b_{li+1}  (PE matvec + add)."""
            psv = psvp.tile([128, 1], F32, tag="psv", name=f"psv{li}_{b}")
            nc.tensor.matmul(psv[:], W(2 * li + 3), m_bf[:],
                             start=True, stop=True)
            vec = vecp.tile([128, 1], F32, tag="v", name=f"vec{li + 1}_{b}")
            nc.vector.tensor_scalar_add(vec, psv[:], bcol(BC_B1 + li))
            return vec

        def run_layer(li, b, lts, tt):
            """Emit the 8 supertiles + lvl1 tree chasers, with fillers."""
            for i in range(NST):
                lts[i]()
                if i % 2 == 1:
                    tt[i // 2]()

        # prologue: pos(0) interleaved with L0(0)
        st0 = new_batch_state(0)
        load_wm()
        states = {0: st0}
        p0 = pos_tasks(0, st0[0], st0[1])
        r1_0 = netp.tile([128, T], BF16, tag="net", name="r1_0")
        l0_0 = layer_tasks(0, 0, st0[1], r1_0, [bcol(BC_B0)])
        tt0, tail0, res0 = tree_tasks(r1_0, 0, 0)
        for t in p0[0:4]:
            t()
        l0_0[0](); l0_0[2]()
        for t in p0[4:8]:
            t()
        l0_0[1](); l0_0[3]()
        tt0[0](); tt0[1]()
        for t in p0[8:12]:
            t()
        l0_0[4](); l0_0[6]()
        for t in p0[12:16]:
            t()
        l0_0[5](); l0_0[7]()
        tt0[2](); tt0[3]()

        for b in range(BPC):
            _, r0 = states[b]

            if b == 0:
                tail, res = tail0, res0
                r1 = r1_0
            else:
                r1 = netp.tile([128, T], BF16, tag="net", name=f"r1_{b}")
                lts = layer_tasks(0, b, r0, r1, [bcol(BC_B0)])
                tt, tail, res = tree_tasks(r1, b, 0)
                run_layer(0, b, lts, tt)
            tail()
            m0 = res["m"]

            filler = []
            if b + 1 < BPC:
                stn = new_batch_state(b + 1)
                states[b + 1] = stn
                filler = pos_tasks(b + 1, stn[0], stn[1])
            for t in filler[0:2]:
                t()

            vec1 = boundary(0, b, m0)

            # L1 + tree
            r2 = netp.tile([128, T], BF16, tag="net", name=f"r2_{b}")
            l1t = layer_tasks(1, b, r1, r2, [vec1])
            tt1, tail1, res1 = tree_tasks(r2, b, 1)
            seq = []
            for i in range(NST):
                seq.append(l1t[i])
                if i % 2 == 1:
                    seq.append(tt1[i // 2])
            for t in interleave(seq, filler[2:4]):
                t()
            tail1()
            m1 = res1["m"]
            for t in filler[4:6]:
                t()
            vec2 = boundary(1, b, m1)

            # L2 + tree
            r3 = netp.tile([128, T], BF16, tag="net", name=f"r3_{b}")
            l2t = layer_tasks(2, b, r2, r3, [vec2])
            tt2, tail2, res2 = tree_tasks(r3, b, 2)
            seq = []
            for i in range(NST):
                seq.append(l2t[i])
                if i % 2 == 1:
                    seq.append(tt2[i // 2])
            for t in interleave(seq, filler[6:10]):
                t()
            tail2()
            m2 = res2["m"]

            vec3 = boundary(2, b, m2)

            # D layer
            partD = smallp.tile([128, NST], F32, tag="pp", name=f"pD_{b}")
            dts = d_tasks(b, r3, partD)
            for t in interleave(dts, filler[10:16]):
                t()

            # final: s = relu(max(partD) + vec3); head
            pmaxD = vecp.tile([128, 1], F32, tag="v", name=f"pmaxD_{b}")
            nc.vector.reduce_max(pmaxD, partD[:, 0:NST], axis=AX)
            hb = vecp.tile([128, 1], BF16, tag="vb", name=f"s_{b}")
            nc.vector.tensor_scalar(hb, pmaxD[:], vec3[:], 0.0,
                                    op0=ADD, op1=MAX)
            for wi, bi in ((8, BC_BC), (9, BC_BM0), (10, BC_BM1), (11, BC_BM2)):
                ps = psvp.tile([128, 1], F32, tag="psv", name=f"ph{wi}_{b}")
                nc.tensor.matmul(ps[:], W(wi), hb[:], start=True, stop=True)
                h2 = vecp.tile([128, 1], BF16, tag="vb", name=f"h{wi}_{b}")
                nc.vector.tensor_scalar(h2, ps[:], bcol(bi), 0.0,
                                        op0=ADD, op1=MAX)
                hb = h2
            ps9 = psvp.tile([9, 1], F32, tag="psv", name=f"po_{b}")
            nc.tensor.matmul(ps9[:], wm_sb[:, 1536:1536 + 9], hb[:],
                             start=True, stop=True)
            ob = headp.tile([9, 1], F32, tag="o", name=f"ob_{b}")
            nc.scalar.activation(ob, ps9[:], IDENT,
                                 bias=bias_sb[0:9, BC_BP:BC_BP + 1])
            nc.sync.dma_start(out_d[b:b + 1].rearrange("b f -> f b"), ob[:])

    for p in reversed(ctx_pools):
        p.release()


def build_program(reps=1):
    nc = bacc.Bacc("TRN2", target_bir_lowering=False, debug=False,
                   num_devices=NCORES)
    pt_d = nc.dram_tensor("pt", [BPC, NQ, 3, QT], BF16, kind="ExternalInput").ap()
    wpos_d = nc.dram_tensor("wpos", [NQ, 3, 256], BF16, kind="ExternalInput").ap()
    wm_d = nc.dram_tensor("wm", [128, WM_COLS], BF16, kind="ExternalInput").ap()
    bias_d = nc.dram_tensor("bias", [128, 16], F32, kind="ExternalInput").ap()
    out_d = nc.dram_tensor("out", [BPC, 9], F32, kind="ExternalOutput").ap()
    with tile.TileContext(nc) as tc:
        _emit_core_program(tc, nc, pt_d, wpos_d, wm_d, bias_d, out_d, reps=reps)
    nc.compile()
    return nc


def prepare_host_inputs(inputs):
    """Shared (weights) and per-core (points) host-side packing."""
    import ml_dtypes
    BF = ml_dtypes.bfloat16
    f = lambda k: np.ascontiguousarray(np.asarray(inputs[k], np.float32))
    p = f("p")
    W_pos, b_pos = f("W_pos"), f("b_pos")
    W0, b0 = f("W0"), f("b0")
    W1, b1 = f("W1"), f("b1")
    W2, b2 = f("W2"), f("b2")
    W3, b3 = f("W3"), f("b3")
    Wc, bc = f("Wc"), f("bc")
    Wm0, bm0 = f("Wm0"), f("bm0")
    Wm1, bm1 = f("Wm1"), f("bm1")
    Wm2, bm2 = f("Wm2"), f("bm2")
    Wp, bp = f("Wp"), f("bp")

    wpos = np.broadcast_to(W_pos, (NQ, 3, 256)).astype(BF)

    wm = np.zeros((128, WM_COLS), np.float32)
    blocks = [W0[:128], W0[128:], W1[:128], W1[128:], W2[:128], W2[128:],
              W3[:128], W3[128:], Wc, Wm0, Wm1, Wm2]
    for i, blk in enumerate(blocks):
        wm[:, 128 * i:128 * (i + 1)] = blk
    wm[:, 1536:1536 + 9] = Wp
    wm = wm.astype(BF)

    bias = np.zeros((128, 16), np.float32)
    bias[:, BC_BPOS_A] = b_pos[:128]
    bias[:, BC_BPOS_B] = b_pos[128:]
    bias[:, BC_B0] = b0
    bias[:, BC_B1] = b1
    bias[:, BC_B2] = b2
    bias[:, BC_B3] = b3
    bias[:, BC_BC] = bc
    bias[:, BC_BM0] = bm0
    bias[:, BC_BM1] = bm1
    bias[:, BC_BM2] = bm2
    bias[:9, BC_BP] = bp

    shared = {"wpos": wpos, "wm": wm, "bias": bias}

    in_maps = []
    for core in range(NCORES):
        pc = p[core * BPC:(core + 1) * BPC]          # [BPC, T, 3]
        pt = np.empty((BPC, NQ, 3, QT), BF)
        for b in range(BPC):
            for q in range(NQ):
                pt[b, q] = pc[b, q * QT:(q + 1) * QT, :].T.astype(BF)
        in_maps.append({"pt": pt, **shared})
    return in_maps


_PROGRAM_CACHE = {}


def kernel(**inputs):
    reps = 1
    if reps not in _PROGRAM_CACHE:
        _PROGRAM_CACHE[reps] = build_program(reps)
    nc = _PROGRAM_CACHE[reps]
    in_maps = prepare_host_inputs(inputs)
    res = run_bass_kernel_spmd(nc, in_maps, core_ids=list(range(NCORES)))
    out = np.concatenate([res.results[i]["out"] for i in range(NCORES)], axis=0)
    return out.astype(np.float32)
